# revision 1
# baseline (speedup 1.0000x reference)
"""Trainium2 Bass kernel for nn_ArchDecoder: two stacked LSTMs (H=2048, H=4096)
unrolled DEPTH=12 sequential steps, batch=1, tensor-parallel across 8 NeuronCores.

Sharding: core k owns h_a positions [256k,256k+256), h_sum positions [256k,+256),
and the HP-LSTM state slice = union of those two regions (so prev_h_hp is local).
Weights are pre-permuted/sharded on host, cast to bf16, and
stay SBUF-resident across all 12 steps. Per step ONE merged AllGather exchanges
[c_hp | h_hp | hp-logit-partials] of step t plus [h_a | arch-logit-partials] of
step t+1 (the arch recurrence is self-contained, so it runs one step ahead).
Output-layer biases are folded in as b/8 per logit partial. Payloads are
partition-major [128, C] blocks so every DMA is the canonical spray pattern; the
gathered rank blocks are read back as one [128, 8*C] tile and the matvec rhs
chunks index its strided columns directly.
"""
import sys

for _p in ("/opt/trn_rl_repo", "/root/.axon_site", "/root/.axon_site/_ro/pypackages"):
    if _p not in sys.path:
        sys.path.insert(0, _p)

import numpy as np
import ml_dtypes

import concourse.bass as bass
import concourse.bacc as bacc
import concourse.mybir as mybir
import concourse.tile as tile
from concourse import bass_isa
from concourse.bass_utils import run_bass_kernel_spmd

NC = 8
V = 256
HA = 2048
HHP = 4096
DEPTH = 12
BF = mybir.dt.bfloat16
F32 = mybir.dt.float32
FP8 = mybir.dt.float8e4
WSC = 32.0   # fp8 weight scale for W_hh_hp
ASC = 16.0   # fp8 activation scale for c_hp
AF = mybir.ActivationFunctionType

SA = HA // NC          # 256 h_a positions per core
SS = HA // NC          # 256 h_sum positions per core
SHP = SA + SS          # 512 hp-state positions per core
MA = 4 * SA // 128     # 8  M-tiles for arch gates
MHP = 4 * SHP // 128   # 16 M-tiles for hp gates
KA = (V + HA) // 128   # 18 K-chunks for arch gates ([a ; h_a])
KHP_C = HHP // 128     # 32 c_hp K-chunks
KHP_I = (2 * V) // 128 # 4 inp K-chunks
KSUM = HHP // 128      # 32
CM = 14                # merged payload cols: [c(4) | h(4) | lpB(2) | h_a(2) | lpA(2)]


def _build_nc():
    nc = bacc.Bacc(None, target_bir_lowering=False, num_devices=NC)

    wa_e = nc.declare_dram_parameter("wa", [128, MA * KA * 128], BF, isOutput=False)
    wsum_e = nc.declare_dram_parameter("wsum", [128, 2 * KSUM * 128], BF, isOutput=False)
    whpc_e = nc.declare_dram_parameter("whpc", [128, MHP * KHP_C * 128], BF, isOutput=False)
    whpi_e = nc.declare_dram_parameter("whpi", [128, MHP * KHP_I * 128], BF, isOutput=False)
    woa_e = nc.declare_dram_parameter("woa", [128, 2 * 2 * 128], BF, isOutput=False)
    wohp_e = nc.declare_dram_parameter("wohp", [128, 2 * 4 * 128], BF, isOutput=False)
    ba_e = nc.declare_dram_parameter("ba", [128, MA], F32, isOutput=False)
    bsum_e = nc.declare_dram_parameter("bsum", [128, 2], F32, isOutput=False)
    bhp_e = nc.declare_dram_parameter("bhp", [128, MHP], F32, isOutput=False)
    boa8_e = nc.declare_dram_parameter("boa8", [128, 2], F32, isOutput=False)
    bohp8_e = nc.declare_dram_parameter("bohp8", [128, 2], F32, isOutput=False)
    init0_e = nc.declare_dram_parameter("init0", [128, NC * CM], BF, isOutput=False)
    out_e = nc.declare_dram_parameter("out", [2, DEPTH, V], F32, isOutput=True)

    with tile.TileContext(nc, num_cores=NC) as tc:
        with (
            tc.tile_pool(name="wpool", bufs=1) as wpool,
            tc.tile_pool(name="cpool", bufs=1) as cpool,
            tc.tile_pool(name="spool", bufs=3) as spool,
            tc.tile_pool(name="xpool", bufs=3) as xpool,
            tc.tile_pool(name="psA", bufs=2, space="PSUM") as psA,
            tc.tile_pool(name="psHP", bufs=2, space="PSUM") as psHP,
            tc.tile_pool(name="psHP2", bufs=2, space="PSUM") as psHP2,
            tc.tile_pool(name="psM", bufs=2, space="PSUM") as psM,
            tc.tile_pool(name="dram", bufs=2, space="DRAM") as dram,
        ):
            wa = wpool.tile([128, MA * KA * 128], BF, tag="wa")
            wsum = wpool.tile([128, 2 * KSUM * 128], BF, tag="wsum")
            whpc = wpool.tile([128, MHP * KHP_C * 128], BF, tag="whpc")
            whpi = wpool.tile([128, MHP * KHP_I * 128], BF, tag="whpi")
            woa = wpool.tile([128, 2 * 2 * 128], BF, tag="woa")
            wohp = wpool.tile([128, 2 * 4 * 128], BF, tag="wohp")
            nc.sync.dma_start(wa[:], wa_e[:])
            nc.sync.dma_start(wsum[:], wsum_e[:])
            nc.sync.dma_start(whpc[:], whpc_e[:])
            nc.sync.dma_start(whpi[:], whpi_e[:])
            nc.sync.dma_start(woa[:], woa_e[:])
            nc.sync.dma_start(wohp[:], wohp_e[:])
            ba = cpool.tile([128, MA], F32, tag="ba")
            bsum = cpool.tile([128, 2], F32, tag="bsum")
            bhp = cpool.tile([128, MHP], F32, tag="bhp")
            boa8 = cpool.tile([128, 2], F32, tag="boa8")
            bohp8 = cpool.tile([128, 2], F32, tag="bohp8")
            nc.sync.dma_start(ba[:], ba_e[:])
            nc.sync.dma_start(bsum[:], bsum_e[:])
            nc.sync.dma_start(bhp[:], bhp_e[:])
            nc.sync.dma_start(boa8[:], boa8_e[:])
            nc.sync.dma_start(bohp8[:], bohp8_e[:])

            ones_c = cpool.tile([128, 1], F32, tag="ones_c")
            ones_r = cpool.tile([1, 128], F32, tag="ones_r")
            nc.vector.memset(ones_c[:], 1.0)
            nc.vector.memset(ones_r[:], 1.0)
            c_a = cpool.tile([128, 2], F32, tag="c_a")
            nc.vector.memset(c_a[:], 0.0)
            outA = cpool.tile([128, 2 * DEPTH], F32, tag="outA")
            outHP = cpool.tile([128, 2 * DEPTH], F32, tag="outHP")

            a_bf0 = xpool.tile([128, 2], BF, tag="a_bf")
            ahp_bf0 = xpool.tile([128, 2], BF, tag="ahp_bf")
            nc.vector.memset(a_bf0[:], 1.0 / V)
            nc.vector.memset(ahp_bf0[:], 1.0 / V)
            init0 = xpool.tile([128, NC * CM], BF, tag="allM")
            nc.sync.dma_start(init0[:], init0_e[:])

            ccol = lambda kc: (kc // 4) * CM + (kc % 4)
            hcol = lambda kc: (kc // 4) * CM + 4 + (kc % 4)
            acol = lambda j: (j // 2) * CM + 10 + (j % 2)

            Ast = {}
            Bst = {}

            def A_mm(t, a_bf, allM):
                ga_ps = psA.tile([128, MA], F32, tag="ga_ps")
                order = list(range(2, KA)) + [0, 1]   # h_a chunks first, a chunks last
                for m in range(MA):
                    for kc in order:
                        if kc < 2:
                            rhs = a_bf[:, kc:kc + 1]
                        else:
                            c = acol(kc - 2)
                            rhs = allM[:, c:c + 1]
                        nc.tensor.matmul(
                            ga_ps[:, m:m + 1],
                            wa[:, (m * KA + kc) * 128:(m * KA + kc + 1) * 128],
                            rhs, start=(kc == 2), stop=(kc == 1),
                        )
                Ast[('ga', t)] = ga_ps

            def A_acts(t):
                ga_ps = Ast.pop(('ga', t))
                acts_a = spool.tile([128, MA], F32, tag="acts_a")
                ga_sb = spool.tile([128, MA], F32, tag="ga_sb")
                nc.vector.tensor_add(ga_sb[:], ga_ps[:], ba[:])
                nc.scalar.activation(acts_a[:, 0:2], ga_sb[:, 0:2], AF.Sigmoid)
                nc.scalar.activation(acts_a[:, 2:4], ga_sb[:, 2:4], AF.Sigmoid)
                nc.scalar.activation(acts_a[:, 6:8], ga_sb[:, 6:8], AF.Sigmoid)
                nc.scalar.activation(acts_a[:, 4:6], ga_sb[:, 4:6], AF.Tanh)
                t1a = spool.tile([128, 2], F32, tag="t1a")
                t2a = spool.tile([128, 2], F32, tag="t2a")
                nc.vector.tensor_mul(t1a[:], acts_a[:, 0:2], acts_a[:, 4:6])
                nc.vector.tensor_mul(t2a[:], acts_a[:, 2:4], c_a[:])
                nc.vector.tensor_add(c_a[:], t1a[:], t2a[:])
                tc_a = spool.tile([128, 2], F32, tag="tc_a")
                nc.scalar.activation(tc_a[:], c_a[:], AF.Tanh)
                ha_f = spool.tile([128, 2], F32, tag="ha_f")
                nc.vector.tensor_mul(ha_f[:], acts_a[:, 6:8], tc_a[:])
                Ast[t] = ha_f
                return ha_f

            def A_fill(t, payM):
                """Write A(t)'s sections into payM cols 10:14."""
                ha_f = Ast[t]
                nc.vector.tensor_copy(payM[:, 10:12], ha_f[:])
                la_ps = psM.tile([128, 2], F32, tag="psM")
                for m in range(2):
                    for kc in range(2):
                        nc.tensor.matmul(
                            la_ps[:, m:m + 1],
                            woa[:, (m * 2 + kc) * 128:(m * 2 + kc + 1) * 128],
                            payM[:, 10 + kc:11 + kc], start=(kc == 0), stop=(kc == 1),
                        )
                nc.vector.tensor_add(payM[:, 12:14], la_ps[:], boa8[:])

            def do_gather(payM, last=False):
                ccM = dram.tile([128, CM], BF, tag="ccM")
                gM = dram.tile([NC, 128, CM], BF, tag="gM")
                nc.sync.dma_start(ccM[:], payM[:])
                nc.gpsimd.collective_compute(
                    "AllGather", mybir.AluOpType.bypass,
                    replica_groups=[list(range(NC))],
                    ins=[ccM.opt()], outs=[gM.opt()],
                )
                allM = xpool.tile([128, NC * CM], BF, tag="allM")
                nc.scalar.dma_start(allM[:], gM[:].rearrange("r p c -> p r c"))
                return allM

            def A_soft(t, allM):
                la_red = spool.tile([128, 2], F32, tag="la_red")
                nc.vector.tensor_reduce(
                    la_red[:],
                    allM[:].rearrange("p (r c) -> p c r", r=NC)[:, 12:14, :],
                    mybir.AxisListType.X, mybir.AluOpType.add,
                )
                exp_a = spool.tile([128, 2], F32, tag="exp_a")
                nc.scalar.activation(exp_a[:], la_red[:], AF.Exp)
                sum_a = spool.tile([128, 2], F32, tag="sum_a")
                nc.gpsimd.partition_all_reduce(sum_a[:], exp_a[:], 128, bass_isa.ReduceOp.add)
                tot_a = spool.tile([128, 1], F32, tag="tot_a")
                nc.vector.tensor_add(tot_a[:], sum_a[:, 0:1], sum_a[:, 1:2])
                rcp_a = spool.tile([128, 1], F32, tag="rcp_a")
                nc.vector.reciprocal(rcp_a[:], tot_a[:])
                nc.vector.tensor_scalar_mul(outA[:, 2 * t:2 * t + 2], exp_a[:], rcp_a[:, 0:1])
                a_bf = xpool.tile([128, 2], BF, tag="a_bf")
                nc.vector.tensor_copy(a_bf[:], outA[:, 2 * t:2 * t + 2])
                return a_bf

            def B_early(t, allM):
                hs_ps = psM.tile([128, 2], F32, tag="psM")
                for m in range(2):
                    for kc in range(KSUM):
                        c = hcol(kc)
                        nc.tensor.matmul(
                            hs_ps[:, m:m + 1],
                            wsum[:, (m * KSUM + kc) * 128:(m * KSUM + kc + 1) * 128],
                            allM[:, c:c + 1], start=(kc == 0), stop=(kc == KSUM - 1),
                        )
                hs_sb = spool.tile([128, 2], F32, tag="hs_sb")
                nc.scalar.activation(hs_sb[:, 0:1], hs_ps[:, 0:1], AF.Relu, bias=bsum[:, 0:1])
                nc.scalar.activation(hs_sb[:, 1:2], hs_ps[:, 1:2], AF.Relu, bias=bsum[:, 1:2])
                ghp_ps = None
                if t > 0:
                    ghp_ps = psHP.tile([128, MHP], F32, tag="ghp_ps")
                    for m in range(MHP):
                        for kc in range(KHP_C):
                            c = ccol(kc)
                            nc.tensor.matmul(
                                ghp_ps[:, m:m + 1],
                                whpc[:, (m * KHP_C + kc) * 128:(m * KHP_C + kc + 1) * 128],
                                allM[:, c:c + 1], start=(kc == 0), stop=(kc == KHP_C - 1),
                            )
                Bst[('early', t)] = (hs_sb, ghp_ps)

            def B_inp(t, a_bf, ahp_bf):
                ghp2_ps = psHP2.tile([128, MHP], F32, tag="ghp2_ps")
                for m in range(MHP):
                    for j in (2, 3, 0, 1):   # a_hp chunks first, a chunks last
                        rhs = a_bf[:, j:j + 1] if j < 2 else ahp_bf[:, j - 2:j - 1]
                        nc.tensor.matmul(
                            ghp2_ps[:, m:m + 1],
                            whpi[:, (m * KHP_I + j) * 128:(m * KHP_I + j + 1) * 128],
                            rhs, start=(j == 2), stop=(j == 1),
                        )
                Bst[('inp', t)] = ghp2_ps

            def B_rest(t):
                """HP acts + payload cols 0:10; returns payM."""
                hs_sb, ghp_ps = Bst.pop(('early', t))
                ghp2_ps = Bst.pop(('inp', t))
                ha_f = Ast.pop(t)
                ghp_sb = spool.tile([128, MHP], F32, tag="ghp_sb")
                acts_h = spool.tile([128, MHP], F32, tag="acts_h")
                if t > 0:
                    gsum = spool.tile([128, MHP], F32, tag="gsum")
                    nc.vector.tensor_add(gsum[:], ghp_ps[:], bhp[:])
                    nc.vector.tensor_add(ghp_sb[:], gsum[:], ghp2_ps[:])
                else:
                    nc.vector.tensor_add(ghp_sb[:], ghp2_ps[:], bhp[:])
                nc.scalar.activation(acts_h[:, 0:4], ghp_sb[:, 0:4], AF.Sigmoid)
                nc.scalar.activation(acts_h[:, 4:8], ghp_sb[:, 4:8], AF.Sigmoid)
                nc.scalar.activation(acts_h[:, 12:16], ghp_sb[:, 12:16], AF.Sigmoid)
                nc.scalar.activation(acts_h[:, 8:12], ghp_sb[:, 8:12], AF.Tanh)
                t1h = spool.tile([128, 4], F32, tag="t1h")
                t2h = spool.tile([128, 4], F32, tag="t2h")
                nc.vector.tensor_mul(t1h[:], acts_h[:, 0:4], acts_h[:, 8:12])
                nc.vector.tensor_mul(t2h[:, 0:2], acts_h[:, 4:6], ha_f[:])
                nc.vector.tensor_mul(t2h[:, 2:4], acts_h[:, 6:8], hs_sb[:])
                con_f = spool.tile([128, 4], F32, tag="con_f")
                nc.vector.tensor_add(con_f[:], t1h[:], t2h[:])
                payM = spool.tile([128, CM], BF, tag="payM")
                nc.vector.tensor_copy(payM[:, 0:4], con_f[:])
                tc_h = spool.tile([128, 4], F32, tag="tc_h")
                nc.scalar.activation(tc_h[:], con_f[:], AF.Tanh)
                nc.vector.tensor_mul(payM[:, 4:8], acts_h[:, 12:16], tc_h[:])
                lhp_ps = psM.tile([128, 2], F32, tag="psM")
                for m in range(2):
                    for kc in range(4):
                        nc.tensor.matmul(
                            lhp_ps[:, m:m + 1],
                            wohp[:, (m * 4 + kc) * 128:(m * 4 + kc + 1) * 128],
                            payM[:, 4 + kc:5 + kc], start=(kc == 0), stop=(kc == 3),
                        )
                nc.vector.tensor_add(payM[:, 8:10], lhp_ps[:], bohp8[:])
                return payM

            def B_soft(t, allM):
                lh_red = spool.tile([128, 2], F32, tag="lh_red")
                nc.vector.tensor_reduce(
                    lh_red[:],
                    allM[:].rearrange("p (r c) -> p c r", r=NC)[:, 8:10, :],
                    mybir.AxisListType.X, mybir.AluOpType.add,
                )
                exp_h = spool.tile([128, 2], F32, tag="exp_h")
                nc.scalar.activation(exp_h[:], lh_red[:], AF.Exp)
                sum_h = spool.tile([128, 2], F32, tag="sum_h")
                nc.gpsimd.partition_all_reduce(sum_h[:], exp_h[:], 128, bass_isa.ReduceOp.add)
                tot_h = spool.tile([128, 1], F32, tag="tot_h")
                nc.vector.tensor_add(tot_h[:], sum_h[:, 0:1], sum_h[:, 1:2])
                rcp_h = spool.tile([128, 1], F32, tag="rcp_h")
                nc.vector.reciprocal(rcp_h[:], tot_h[:])
                nc.vector.tensor_scalar_mul(outHP[:, 2 * t:2 * t + 2], exp_h[:], rcp_h[:, 0:1])
                ahp_bf = xpool.tile([128, 2], BF, tag="ahp_bf")
                nc.vector.tensor_copy(ahp_bf[:], outHP[:, 2 * t:2 * t + 2])
                return ahp_bf

            # ---- pipelined emission: one merged gather per step; A one step ahead ----
            A_mm(0, a_bf0, init0)
            A_acts(0)
            payA0 = spool.tile([128, CM], BF, tag="payM")
            A_fill(0, payA0)
            Mprev = do_gather(payA0, last=True)   # delivers allA(0); no B sections
            B_early(0, init0)
            abf = A_soft(0, Mprev)
            B_inp(0, abf, ahp_bf0)
            payM = B_rest(0)
            A_mm(1, abf, Mprev)
            A_acts(1)
            A_fill(1, payM)
            M = do_gather(payM)                    # M[0]: allB(0) + allA(1)
            Mprev = M
            for t in range(1, DEPTH):
                ahp_bf = B_soft(t - 1, Mprev)
                B_early(t, Mprev)
                abf = A_soft(t, Mprev)
                B_inp(t, abf, ahp_bf)
                payM = B_rest(t)
                if t + 1 < DEPTH:
                    A_mm(t + 1, abf, Mprev)
                    A_acts(t + 1)
                    A_fill(t + 1, payM)
                    Mprev = do_gather(payM)
                else:
                    Mprev = do_gather(payM, last=True)
            B_soft(DEPTH - 1, Mprev)

            for t in range(DEPTH):
                nc.sync.dma_start(
                    out_e[0, t].rearrange("(m p) -> p m", p=128),
                    outA[:, 2 * t:2 * t + 2],
                )
                nc.sync.dma_start(
                    out_e[1, t].rearrange("(m p) -> p m", p=128),
                    outHP[:, 2 * t:2 * t + 2],
                )
    nc.finalize()
    return nc


_NC_CACHE = None


def _get_nc():
    global _NC_CACHE
    if _NC_CACHE is None:
        _NC_CACHE = _build_nc()
    return _NC_CACHE


def _lhsT_pack(w_cat, n_m, n_k):
    """w_cat [n_m*128 rows, n_k*128 cols] -> SBUF image [128, n_m*n_k*128] where
    cols [(m*n_k+kc)*128 + j] on partition p = w_cat[m*128 + j, kc*128 + p]."""
    a = w_cat.reshape(n_m, 128, n_k, 128)           # [m, j, kc, p]
    return np.ascontiguousarray(a.transpose(3, 0, 2, 1).reshape(128, n_m * n_k * 128))


def _prep_in_maps(x_thought_vec_arch, x_thought_vec_arch_hp,
                  W_ih_a, W_hh_a, b_ih_a, b_hh_a, W_out_a, b_out_a,
                  W_sum, b_sum, W_ih_hp, W_hh_hp, b_ih_hp, b_hh_hp,
                  W_out_hp, b_out_hp):
    f32 = np.float32
    bf16 = ml_dtypes.bfloat16
    php = np.concatenate([
        np.concatenate([np.arange(SA * k, SA * (k + 1)),
                        HA + np.arange(SS * k, SS * (k + 1))])
        for k in range(NC)
    ])
    ba_full = (np.asarray(b_ih_a) + np.asarray(b_hh_a)).astype(f32)
    bhp_full = (np.asarray(b_ih_hp) + np.asarray(b_hh_hp)).astype(f32)
    ha0 = np.asarray(x_thought_vec_arch, f32).reshape(HA)
    hhp0 = np.asarray(x_thought_vec_arch_hp, f32).reshape(HHP)
    W_ih_a = np.asarray(W_ih_a, f32); W_hh_a = np.asarray(W_hh_a, f32)
    W_out_a = np.asarray(W_out_a, f32); W_sum = np.asarray(W_sum, f32)
    W_ih_hp = np.asarray(W_ih_hp, f32); W_hh_hp = np.asarray(W_hh_hp, f32)
    W_out_hp = np.asarray(W_out_hp, f32)
    b_out_a = np.asarray(b_out_a, f32); b_out_hp = np.asarray(b_out_hp, f32)
    b_sum = np.asarray(b_sum, f32)

    init0 = np.zeros((128, NC * CM), f32)
    hhp0_p = hhp0[php]
    for r in range(NC):
        for q in range(4):
            init0[:, r * CM + 4 + q] = hhp0_p[r * SHP + q * 128: r * SHP + (q + 1) * 128]
        for m in range(2):
            init0[:, r * CM + 10 + m] = ha0[r * SA + m * 128: r * SA + (m + 1) * 128]

    in_maps = []
    for k in range(NC):
        ja = np.arange(SA * k, SA * (k + 1))
        rows_a = np.concatenate([g * HA + ja for g in range(4)])
        wa_cat = np.concatenate([W_ih_a[rows_a], W_hh_a[rows_a]], axis=1)
        jhp = php[SHP * k: SHP * (k + 1)]
        rows_hp = np.concatenate([g * HHP + jhp for g in range(4)])
        whpc_cat = W_hh_hp[rows_hp][:, php]
        whpi_cat = W_ih_hp[rows_hp]
        js = np.arange(SS * k, SS * (k + 1))
        wsum_p = W_sum[js][:, php]
        woa_p = W_out_a[:, ja]
        wohp_p = W_out_hp[:, jhp]
        in_maps.append({
            "wa": _lhsT_pack(wa_cat, MA, KA).astype(bf16),
            "wsum": _lhsT_pack(wsum_p, 2, KSUM).astype(bf16),
            "whpc": _lhsT_pack(whpc_cat, MHP, KHP_C).astype(bf16),
            "whpi": _lhsT_pack(whpi_cat, MHP, KHP_I).astype(bf16),
            "woa": _lhsT_pack(woa_p, 2, 2).astype(bf16),
            "wohp": _lhsT_pack(wohp_p, 2, 4).astype(bf16),
            "ba": np.ascontiguousarray(ba_full[rows_a].reshape(MA, 128).T),
            "bsum": np.ascontiguousarray(b_sum[js].reshape(2, 128).T),
            "bhp": np.ascontiguousarray(bhp_full[rows_hp].reshape(MHP, 128).T),
            "boa8": np.ascontiguousarray((b_out_a / NC).reshape(2, 128).T),
            "bohp8": np.ascontiguousarray((b_out_hp / NC).reshape(2, 128).T),
            "init0": init0.astype(bf16),
        })
    return in_maps


def _run(in_maps, trace=False):
    nc = _get_nc()
    return run_bass_kernel_spmd(nc, in_maps, core_ids=list(range(NC)), trace=trace)


def kernel(**inputs):
    in_maps = _prep_in_maps(**{k: np.asarray(v) for k, v in inputs.items()})
    res = _run(in_maps, trace=False)
    out = np.asarray(res.results[0]["out"], np.float32)
    return out[0][None], out[1][None]


def kernel_traced(**inputs):
    """Like kernel() but with NTFF profiling; returns ((arch, arch_hp), exec_time_ns)."""
    try:
        import ntff_hook
        ntff_hook.install()
    except Exception:
        pass
    in_maps = _prep_in_maps(**{k: np.asarray(v) for k, v in inputs.items()})
    res = _run(in_maps, trace=True)
    out = np.asarray(res.results[0]["out"], np.float32)
    return (out[0][None], out[1][None]), res.exec_time_ns

